# revision 39
# baseline (speedup 1.0000x reference)
"""DecorrelatedBN (ZCA whitening) Trainium2 Bass kernel — 8-core data-parallel.

Problem: x [64,32,32,512] f32, NHWC, channel groups of m=64 (G=8 groups).
  out = ((x - mean) @ P) * gamma + beta,  P = (sigma + eps*I)^(-1/2) per group.

v9 design (from v8). Critical-path levers found in the v8 trace:
  - CC-core init barrier (~45us) attaches to the FIRST collective doorbell.
    v8 rang it at ~20.5us (dummy waited on const-DMA chain); v9's dummy
    AllReduce has NO input dependency (uninitialized internal DRAM tile,
    output unused) so the doorbell rings right after the engine preamble
    (~10us) and the barrier fully overlaps phase A.
  - Each sub-256KB AllReduce is latency-bound (~14-15us on the CC stream).
    v8 ran two sigma halves (29us serial); v9 accumulates all 64 row-tiles
    in one PSUM group and runs ONE AllReduce. The partial per-pair mean
    column is also transposed to a row and packed (scaled 1/M) into the
    payload pre-AR, removing one serial post-AR PE round (the mu mu^T
    outer product needs mu as a row).
  - statistics: host pre-tiles an fp8-e4m3 copy of x into the exact SBUF
    stage layout [128p, chunk, super, pair, 2, 144] with a ones column at
    offset 128 (pad to 144 keeps the DoubleRow LDWEIGHTS subtile step%16==0
    ISA rule). Sigma is 128 DoubleRow matmuls and the channel mean
    accumulates in PSUM column 128 for free. Stage chunks post BEFORE the
    xT prefetch so stats never starve (q1 is HBM-bound at ~358GB/s).
  - P = A^(-1/2): deg-4 minimax poly via Paterson-Stockmeyer (2 PE rounds
    vs 4 Horner) + 2 coupled Newton-Schulz (numpy-validated 6.1e-3 vs the
    2e-2 gate; eigenvalues span [0.057, 2.03]).
  - apply: bf16 from host-supplied channel-major xcol, stationary = P' per
    pair, 64 x 512-wide matmuls into [128,2048] PSUM units; each unit's
    eviction is split DVE (cols 0:1024) + ACT (1024:2048) so both engines
    run per unit and PSUM banks free ~1us earlier. Output is transposed
    (yt [512,8192] bf16), host untransposes/upcasts.
"""
import sys

sys.path.insert(0, "/opt/trn_rl_repo")

import numpy as np
import concourse.bass as bass
import concourse.bacc as bacc
import concourse.tile as tile
import concourse.mybir as mybir
from concourse import bass_utils

dt = mybir.dt
Alu = mybir.AluOpType
Act = mybir.ActivationFunctionType
PerfMode = mybir.MatmulPerfMode

VARIANT = "v9"

# Problem constants (hardcoded per harness contract)
N, H, W, C = 64, 32, 32, 512
M_TOTAL = N * H * W          # 65536 rows
N_CORES = 8
M_LOC = M_TOTAL // N_CORES   # 8192 rows per core
GROUP = 64                   # channels per whitening group
N_PAIRS = 4                  # 8 groups packed as 4 pairs of [128,128] blocks
CPAD = 144                   # 128 chans + ones col + pad (step%16==0)

ROWS_PER_TILE = 128
N_TILES = M_LOC // ROWS_PER_TILE      # 64 row-tiles per core
TILES_PER_CHUNK = 16                  # 16 tiles per input DMA
N_CHUNKS = N_TILES // TILES_PER_CHUNK # 4 chunks
V_PER_CHUNK = TILES_PER_CHUNK // 2    # 8 DoubleRow supertiles per chunk
STAGE_FREE = V_PER_CHUNK * N_PAIRS * 2 * CPAD   # 9216 fp8 bytes/partition
N_SUPER = N_CHUNKS * V_PER_CHUNK      # 32 supertiles
NS_ITERS = 2
ROWS_PER_UNIT = 1024                  # apply-phase evict/DMA unit (2 banks)
N_UNITS = M_LOC // ROWS_PER_UNIT      # 8 units per pair

# AR payload layout: [128, AR_COLS] f32
#   cols 0:256    = 8 diag 64x64 sigma blocks (2 per pair, packed)
#   cols 256:260  = per-pair mean columns (raw sums)
AR_COLS = 260

# "all8": 8-rank dummy; "self": per-core groups; "pairs": 2-rank groups
DUMMY_MODE = "self"

# "cc": CC-core AllReduce (init barrier ~45us + ~19us dummy + ~19us AR all
#       serial on the CC stream). "rdma": no collectives at all — each core
#       broadcasts its partial over SDMA to all 7 peers' SBUF (XOR-symmetric
#       slots, one-shot) and sums locally; relies on launch skew << ~20us
#       (observed peer skew at the CC rendezvous is ~5us).
AR_MODE = "rdma"

# degree-4 minimax-relative fit of a^-1/2 on [0.03, 2.4] (Lawson); with
# 2 coupled NS iterations: rel err ~6e-4 on the data eigenvalue range
# [0.057, 2.03] (deterministic seed).
POLY_COEF = [4.858203701346275, -13.706787063800203, 16.713432649944906,
             -8.387599448841533, 1.462158293274531]

_CACHED = {}


def _build_bass():
    nc = bacc.Bacc("TRN2", target_bir_lowering=False, debug=False,
                   num_devices=N_CORES,
                   num_swdge_queues=4 if AR_MODE == "rdma" else 1)
    f32 = dt.float32
    bf16 = dt.bfloat16
    f8 = dt.float8e4

    xstat_t = nc.dram_tensor("xstat_t", [128, N_CHUNKS * STAGE_FREE], f8,
                             kind="ExternalInput").ap()
    xcol = nc.dram_tensor("xcol", [C, M_LOC], bf16, kind="ExternalInput").ap()
    gamma_row = nc.dram_tensor("gamma_row", [1, C], f32, kind="ExternalInput").ap()
    gamma_t = nc.dram_tensor("gamma_t", [128, N_PAIRS], f32, kind="ExternalInput").ap()
    beta_t = nc.dram_tensor("beta_t", [128, N_PAIRS], f32, kind="ExternalInput").ap()
    ident = nc.dram_tensor("ident", [128, 128], f32, kind="ExternalInput").ap()
    eye15 = nc.dram_tensor("eye15", [128, 128], f32, kind="ExternalInput").ap()
    ones_row = nc.dram_tensor("ones_row", [1, 128], f32, kind="ExternalInput").ap()
    yt = nc.dram_tensor("yt", [C, M_LOC], bf16, kind="ExternalOutput").ap()

    if DUMMY_MODE == "self":
        dummy_groups = [[k] for k in range(N_CORES)]
    elif DUMMY_MODE == "pairs":
        dummy_groups = [[2 * k, 2 * k + 1] for k in range(N_CORES // 2)]
    else:
        dummy_groups = [list(range(N_CORES))]

    with tile.TileContext(nc) as tc:
        with (
            tc.tile_pool(name="const", bufs=1) as constp,
            tc.tile_pool(name="resid", bufs=1) as residp,
            tc.tile_pool(name="small", bufs=1) as smallp,
            tc.tile_pool(name="dram", bufs=1, space="DRAM") as dramp,
        ):
            # dummy collective, zero dependencies; output never read so no
            # engine queue blocks on its completion. In cc mode it rings
            # the CC doorbell right after the preamble so the ~45us CC-core
            # init barrier overlaps phase A. In rdma mode it is emitted
            # AFTER the exchange's critical section instead (the section's
            # global-clock entry snapshot would otherwise gate the exchange
            # trigger on the dummy's ~84us CC completion) — still load-
            # bearing there: has_collectives=True makes PJRT gang-launch
            # the 8 cores (otherwise launches stagger ~1ms/core through
            # the axon tunnel and the peer-exchange waits eat the skew).
            dummy_in = dramp.tile([1, N_PAIRS], f32, name="dummy_in")
            dummy_out = dramp.tile([1, N_PAIRS], f32, name="dummy_out")

            def emit_dummy():
                nc.gpsimd.collective_compute(
                    "AllReduce", Alu.add,
                    replica_groups=dummy_groups if AR_MODE == "cc"
                    else [list(range(N_CORES))],
                    ins=[dummy_in.opt()], outs=[dummy_out.opt()],
                )

            if AR_MODE == "cc":
                emit_dummy()

            # ---- constants to SBUF ----
            id_sb = constp.tile([128, 128], f32, name="id_sb")
            eye15_sb = constp.tile([128, 128], f32, name="eye15_sb")
            onesr_sb = constp.tile([1, 128], f32, name="onesr_sb")
            grow_sb = constp.tile([1, C], f32, name="grow_sb")
            gt_sb = constp.tile([128, N_PAIRS], f32, name="gt_sb")
            bt_sb = constp.tile([128, N_PAIRS], f32, name="bt_sb")

            # resident channel-major x, one tile per 128-channel block so
            # phase B's per-pair deps attach to exactly one prefetch DMA
            xT = [residp.tile([128, M_LOC], bf16, name=f"xT{b}")
                  for b in range(N_PAIRS)]

            # AR staging
            sig_full = smallp.tile([128, AR_COLS], f32, name="sig_full")
            ar_in = dramp.tile([128, AR_COLS], f32, name="ar_in")
            ar_out = dramp.tile([128, AR_COLS], f32, name="ar_out")
            sigsum = smallp.tile([128, AR_COLS], f32, name="sigsum")
            c0I_sb = smallp.tile([128, 128], f32, name="c0I_sb")

            # PE warmup: HAM clock-gate needs sustained matmul activity
            warm_sb = constp.tile([128, 512], bf16, name="warm_sb")

            with (
                tc.tile_pool(name="instage", bufs=1) as inp,
                tc.tile_pool(name="sigps", bufs=1, space="PSUM") as sigpp,
            ):
                sig_ps = [sigpp.tile([128, CPAD - 15], f32, name=f"sig{p}",
                                     tag=f"sig{p}") for p in range(N_PAIRS)]
                stages = [inp.tile([128, V_PER_CHUNK, N_PAIRS, 2, CPAD], f8,
                                   name=f"stage{ch}", tag=f"instage{ch}")
                          for ch in range(N_CHUNKS)]

                # post input DMAs up-front: consts go on the SCALAR queue
                # (their many small per-partition descriptors would clog q1
                # ahead of the stats chunks); stats chunks first on q1
                # (phase A is PE-bound and must never starve). The xT posts
                # are emitted AFTER the rdma critical section (in rdma mode)
                # so its entry snapshot does not gate the exchange trigger
                # on the xT loads; they still stream right behind the
                # chunks on q1.
                nc.scalar.dma_start(id_sb[:], ident[:])
                nc.scalar.dma_start(eye15_sb[:], eye15[:])
                nc.scalar.dma_start(onesr_sb[:], ones_row[:])
                nc.scalar.dma_start(grow_sb[:], gamma_row[:])
                nc.scalar.dma_start(gt_sb[:], gamma_t[:])
                nc.scalar.dma_start(bt_sb[:], beta_t[:])
                for ch in range(N_CHUNKS):
                    nc.sync.dma_start(
                        stages[ch][:],
                        xstat_t[:, ch * STAGE_FREE:(ch + 1) * STAGE_FREE])
                if AR_MODE == "cc":
                    for b in range(N_PAIRS):
                        nc.sync.dma_start(xT[b][:],
                                          xcol[b * 128:(b + 1) * 128, :])

                nc.vector.memset(warm_sb[:], 0.5)

                # PE warmup while chunk 0 streams in
                with tc.tile_pool(name="warmps", bufs=1, space="PSUM") as warmpp:
                    warm_ps = warmpp.tile([128, 512], f32, name="warm_ps")
                    for _ in range(12):
                        nc.tensor.matmul(warm_ps[:], warm_sb[:, 0:128],
                                         warm_sb[:], start=True, stop=True)

                # ============ Phase A: stats (fp8 DoubleRow) ============
                # stage layout [128, super, pair, 2, 144]: data cols 0:128,
                # ones col 128 (mean accumulates in sigma PSUM col 128), pad
                # to 144 for the dual-fp8 LDWEIGHTS step%16 rule. ONE PSUM
                # accumulation group over all 32 supertiles.
                for ch in range(N_CHUNKS):
                    for v in range(V_PER_CHUNK):
                        s = ch * V_PER_CHUNK + v
                        for p in range(N_PAIRS):
                            nc.tensor.matmul(
                                sig_ps[p][:],
                                stages[ch][:, v, p, :, 0:128],
                                stages[ch][:, v, p, :, 0:129],
                                start=(s == 0), stop=(s == N_SUPER - 1),
                                perf_mode=PerfMode.DoubleRow)

                # evac: pack diag blocks + raw mean cols into sig_full
                for p in range(N_PAIRS):
                    nc.scalar.copy(sig_full[0:64, p * 64:(p + 1) * 64],
                                   sig_ps[p][0:64, 0:64])
                    nc.scalar.copy(sig_full[64:128, p * 64:(p + 1) * 64],
                                   sig_ps[p][64:128, 64:128])
                    nc.scalar.copy(sig_full[:, 256 + p:257 + p],
                                   sig_ps[p][:, 128:129])

            if AR_MODE == "cc":
                # ONE AllReduce for everything
                nc.scalar.dma_start(ar_in[:], sig_full[:])
                nc.gpsimd.collective_compute(
                    "AllReduce", Alu.add,
                    replica_groups=[list(range(N_CORES))],
                    ins=[ar_in.opt()], outs=[ar_out.opt()],
                )
                nc.scalar.dma_start(sigsum[:], ar_out[:])
            else:
                # one-shot SDMA all-exchange: 7 single-slot relative
                # broadcasts, slot k -> peer (tpb ^ k), landing in
                # recv[k-1] (XOR-symmetric: at receiver q, slot k's writer
                # is q^k, so every peer lands in a distinct slot; slot k
                # naturally satisfies the D2D-lane rule). Direct one-hop
                # paths are optimal under launch skew (~50us cross-die):
                # core 0's wait = max over peers of skew + peer-stats +
                # transfer; a multi-round ring relays the slowest core's
                # data through extra hops. All control on gpsimd inside a
                # critical section (Tile would otherwise gate the adds on
                # DESC-GEN completion, which merely looks like the writer).
                rd_mono = nc.monotonic_semaphore(0)
                rd_local = nc.alloc_semaphore("rd_local_sem")
                rd_prep = nc.alloc_semaphore("rd_prep_sem")
                recv = [smallp.tile([128, AR_COLS], f32, name=f"rdrecv{k}")
                        for k in range(1, N_CORES)]
                with tc.tile_critical(name="rdma_ar"):
                    for k in range(1, N_CORES):
                        rdests = [None] * N_CORES
                        rdests[k] = (0, k)
                        q = (k - 1) % 4
                        nc.gpsimd.remote_dma_broadcast(
                            recv[k - 1][:], sig_full[:], rd_mono.sem(),
                            rd_local, rdests=rdests,
                            queue_num=q).then_inc(rd_prep, 1)
                    tc.wait_critical_data_deps()
                    nc.gpsimd.wait_ge(rd_prep, N_CORES - 1)
                    for q in range(4):
                        nc.gpsimd.trigger_dma(count=2 if q < 3 else 1,
                                              queue_num=q)
                    # wait for all 7 peers' payloads (2 lane-incs each) on
                    # gpsimd (owns the monotonic counter register), then
                    # gate the DVE reduction (raw order holds in-section)
                    rd_gate = nc.alloc_semaphore("rd_gate_sem")
                    rd_mono.wait_inc(2 * (N_CORES - 1)).then_inc(rd_gate, 1)
                    nc.vector.wait_ge(rd_gate, 1)
                    nc.vector.tensor_tensor(sigsum[:], sig_full[:],
                                            recv[0][:], op=Alu.add)
                    for j in range(1, N_CORES - 1):
                        nc.vector.tensor_tensor(sigsum[:], sigsum[:],
                                                recv[j][:], op=Alu.add)
                # fake dep: write dummy_in from sigsum so Tile schedules
                # the dummy AFTER the exchange (a dep-free collective gets
                # scheduled early and its completion-increment lands in the
                # critical section's entry snapshot, gating the trigger on
                # the ~80us CC dummy completion)
                nc.scalar.dma_start(dummy_in[:], sigsum[0:1, 0:N_PAIRS])
                emit_dummy()
                for b in range(N_PAIRS):
                    nc.sync.dma_start(xT[b][:],
                                      xcol[b * 128:(b + 1) * 128, :])

            # gamma replicate (depends only on gamma): runs during the AR
            grep_sb = smallp.tile([128, C], f32, name="grep_sb")
            with tc.tile_pool(name="grepps", bufs=1, space="PSUM") as greppp:
                grep_ps = greppp.tile([128, C], f32, name="grep_ps")
                nc.tensor.matmul(grep_ps[:], onesr_sb[:], grow_sb[:],
                                 start=True, stop=True)
                nc.scalar.copy(grep_sb[:], grep_ps[:])
            nc.vector.tensor_scalar_mul(c0I_sb[:], id_sb[:],
                                        float(POLY_COEF[0]))

            # keep the PE/HAM clock warm through the AllReduce wait
            with tc.tile_pool(name="warmps2", bufs=1, space="PSUM") as warmpp2:
                warm2_ps = warmpp2.tile([128, 512], f32, name="warm2_ps")
                for _ in range(24):
                    nc.tensor.matmul(warm2_ps[:, 0:256], warm_sb[:, 0:128],
                                     warm_sb[:, 0:256], start=True, stop=True)

            # ================= small-matrix phase (all fp32) ==============
            with tc.tile_pool(name="nsps", bufs=2, space="PSUM") as nspp:
                mu_row = smallp.tile([1, C], f32, name="mu_row")
                mu_col = smallp.tile([128, N_PAIRS], f32, name="mu_col")
                for p in range(N_PAIRS):
                    nc.vector.tensor_scalar_mul(
                        mu_col[:, p:p + 1], sigsum[:, 256 + p:257 + p],
                        1.0 / M_TOTAL)
                    mr_ps = nspp.tile([1, 128], f32, tag="ns2")
                    nc.tensor.matmul(mr_ps[:], mu_col[:, p:p + 1], id_sb[:],
                                     start=True, stop=True)
                    nc.vector.tensor_copy(mu_row[0:1, p * 128:(p + 1) * 128],
                                          mr_ps[:])

                Y_sb = [smallp.tile([128, 128], f32, name=f"Y{p}")
                        for p in range(N_PAIRS)]
                Z_sb = [smallp.tile([128, 128], f32, name=f"Z{p}")
                        for p in range(N_PAIRS)]
                B_sb = [smallp.tile([128, 128], f32, name=f"B{p}")
                        for p in range(N_PAIRS)]
                A_sb = [smallp.tile([128, 128], f32, name=f"A{p}")
                        for p in range(N_PAIRS)]
                A2_sb = [smallp.tile([128, 128], f32, name=f"A2_{p}")
                         for p in range(N_PAIRS)]
                T1_sb = [smallp.tile([128, 128], f32, name=f"T1_{p}")
                         for p in range(N_PAIRS)]
                T0_sb = [smallp.tile([128, 128], f32, name=f"T0_{p}")
                        for p in range(N_PAIRS)]
                Pb_sb = [smallp.tile([128, 128], bf16, name=f"Pb{p}")
                         for p in range(N_PAIRS)]

                # A_p = blockdiag(sigsum/M - mu mu^T)   (eps dropped: <1e-4)
                for p in range(N_PAIRS):
                    murow = mu_row[0:1, p * 128:(p + 1) * 128]
                    outer_ps = nspp.tile([128, 128], f32, tag="ns0")
                    nc.tensor.matmul(outer_ps[:], murow, murow,
                                     start=True, stop=True)
                    nc.vector.memset(A_sb[p][:], 0.0)
                    nc.vector.scalar_tensor_tensor(
                        A_sb[p][0:64, 0:64], sigsum[0:64, p * 64:(p + 1) * 64],
                        1.0 / M_TOTAL, outer_ps[0:64, 0:64],
                        op0=Alu.mult, op1=Alu.subtract)
                    nc.vector.scalar_tensor_tensor(
                        A_sb[p][64:128, 64:128],
                        sigsum[64:128, p * 64:(p + 1) * 64],
                        1.0 / M_TOTAL, outer_ps[64:128, 64:128],
                        op0=Alu.mult, op1=Alu.subtract)

                # polynomial init via Paterson-Stockmeyer (2 PE rounds):
                #   A2 = A@A
                #   T1 = c3*A + c4*A2 ; T0 = c1*A + (c2*A2 + c0*I)
                #   Z0 = T1@A2 + T0
                c1, c2, c3, c4 = (float(POLY_COEF[1]), float(POLY_COEF[2]),
                                  float(POLY_COEF[3]), float(POLY_COEF[4]))
                for p in range(N_PAIRS):
                    a2_ps = nspp.tile([128, 128], f32, tag="ns0")
                    nc.tensor.matmul(a2_ps[:], A_sb[p][:], A_sb[p][:],
                                     start=True, stop=True)
                    nc.vector.tensor_copy(A2_sb[p][:], a2_ps[:])
                for p in range(N_PAIRS):
                    nc.vector.tensor_scalar_mul(T1_sb[p][:], A2_sb[p][:], c4)
                    nc.vector.scalar_tensor_tensor(
                        T1_sb[p][:], A_sb[p][:], c3, T1_sb[p][:],
                        op0=Alu.mult, op1=Alu.add)
                    nc.vector.scalar_tensor_tensor(
                        T0_sb[p][:], A2_sb[p][:], c2, c0I_sb[:],
                        op0=Alu.mult, op1=Alu.add)
                    nc.vector.scalar_tensor_tensor(
                        T0_sb[p][:], A_sb[p][:], c1, T0_sb[p][:],
                        op0=Alu.mult, op1=Alu.add)
                for p in range(N_PAIRS):
                    z_ps = nspp.tile([128, 128], f32, tag="ns1")
                    nc.tensor.matmul(z_ps[:], T1_sb[p][:], A2_sb[p][:],
                                     start=True, stop=True)
                    nc.vector.tensor_tensor(Z_sb[p][:], z_ps[:], T0_sb[p][:],
                                            op=Alu.add)
                # Y0 = A @ Z0
                for p in range(N_PAIRS):
                    y_ps = nspp.tile([128, 128], f32, tag="ns1")
                    nc.tensor.matmul(y_ps[:], A_sb[p][:], Z_sb[p][:],
                                     start=True, stop=True)
                    nc.scalar.copy(Y_sb[p][:], y_ps[:])

                # coupled Newton-Schulz: W=Z@Y; B=1.5I-0.5W; Y=Y@B; Z=B@Z.
                # bias needs mp = Z_final @ mu = B_last @ (Z_prev @ mu): the
                # v = Z@mu matmul rides the last W round and mp = B@v rides
                # the last Z round, so no extra serial bias round.
                bias_col = smallp.tile([128, N_PAIRS], f32, name="bias_col")
                tmp_col = smallp.tile([128, N_PAIRS], f32, name="tmp_col")
                v_col = smallp.tile([128, N_PAIRS], f32, name="v_col")
                for it in range(NS_ITERS):
                    last = it == NS_ITERS - 1
                    for p in range(N_PAIRS):
                        w_ps = nspp.tile([128, 128], f32, tag="ns0")
                        nc.tensor.matmul(w_ps[:], Z_sb[p][:], Y_sb[p][:],
                                         start=True, stop=True)
                        if last:
                            v_ps = nspp.tile([128, 1], f32, tag="ns2")
                            nc.tensor.matmul(v_ps[:], Z_sb[p][:],
                                             mu_col[:, p:p + 1],
                                             start=True, stop=True)
                            nc.vector.tensor_copy(v_col[:, p:p + 1], v_ps[:])
                        nc.vector.scalar_tensor_tensor(
                            B_sb[p][:], w_ps[:], -0.5, eye15_sb[:],
                            op0=Alu.mult, op1=Alu.add)
                    for p in range(N_PAIRS):
                        y_ps = nspp.tile([128, 128], f32, tag="ns1")
                        z_ps = nspp.tile([128, 128], f32, tag="ns2")
                        if not last:
                            nc.tensor.matmul(y_ps[:], Y_sb[p][:], B_sb[p][:],
                                             start=True, stop=True)
                            nc.scalar.copy(Y_sb[p][:], y_ps[:])
                        nc.tensor.matmul(z_ps[:], B_sb[p][:], Z_sb[p][:],
                                         start=True, stop=True)
                        nc.vector.tensor_copy(Z_sb[p][:], z_ps[:])
                        if last:
                            mp_ps = nspp.tile([128, 1], f32, tag="ns0")
                            nc.tensor.matmul(mp_ps[:], B_sb[p][:],
                                             v_col[:, p:p + 1],
                                             start=True, stop=True)
                            nc.vector.tensor_scalar(
                                tmp_col[:, p:p + 1], mp_ps[:],
                                gt_sb[:, p:p + 1], None, op0=Alu.mult)
                            nc.vector.scalar_tensor_tensor(
                                bias_col[:, p:p + 1], tmp_col[:, p:p + 1],
                                -1.0, bt_sb[:, p:p + 1],
                                op0=Alu.mult, op1=Alu.add)

                # gamma fold: P'_bf = Z .* gamma_rep (column scale), bf16
                for p in range(N_PAIRS):
                    nc.vector.tensor_tensor(
                        Pb_sb[p][:], Z_sb[p][:],
                        grep_sb[:, p * 128:(p + 1) * 128], op=Alu.mult)

            # ================= Phase B: apply =================
            # [128,1024] PSUM units (2 banks) x 4 bufs -> 4 units in flight;
            # whole-unit evictions alternate DVE/ACT (PSUM reads run at
            # ~0.5 elem/cyc/engine, so two engines on one unit just split
            # the same bandwidth — alternating engines across units gives
            # the same throughput with fewer dependency edges).
            with (
                tc.tile_pool(name="outstage", bufs=4) as outp,
                tc.tile_pool(name="whps", bufs=4, space="PSUM") as whpp,
            ):
                ucount = 0
                for p in range(N_PAIRS):
                    for un in range(N_UNITS):
                        r0 = un * ROWS_PER_UNIT
                        wh = whpp.tile([128, ROWS_PER_UNIT], f32, tag="whps")
                        for h in range(ROWS_PER_UNIT // 512):
                            nc.tensor.matmul(
                                wh[:, h * 512:(h + 1) * 512],
                                Pb_sb[p][:],
                                xT[p][:, r0 + h * 512: r0 + (h + 1) * 512],
                                start=True, stop=True)
                        ostage = outp.tile([128, ROWS_PER_UNIT], bf16,
                                           tag="outstage")
                        if ucount % 2 == 0:
                            nc.vector.tensor_scalar(
                                ostage[:], wh[:], bias_col[:, p:p + 1], None,
                                op0=Alu.add)
                        else:
                            nc.scalar.activation(
                                ostage[:], wh[:], Act.Identity,
                                bias=bias_col[:, p:p + 1], scale=1.0)
                        nc.sync.dma_start(
                            yt[p * 128:(p + 1) * 128, r0:r0 + ROWS_PER_UNIT],
                            ostage[:])
                        ucount += 1

    nc.compile()
    return nc


def _get_nc():
    if "nc" not in _CACHED:
        _CACHED["nc"] = _build_bass()
    return _CACHED["nc"]


def _const_inputs():
    if "consts" not in _CACHED:
        ident = np.eye(128, dtype=np.float32)
        _CACHED["consts"] = {
            "ident": ident,
            "eye15": (1.5 * ident).astype(np.float32),
            "ones_row": np.ones((1, 128), dtype=np.float32),
        }
    return _CACHED["consts"]


def _pack_xstat(x8k):
    """[8192, 512] fp8 -> pre-tiled stage mirror [128, N_CHUNKS*STAGE_FREE].

    row = ch*2048 + v*256 + i*128 + p; channel = pr*128 + c;
    dest[p, ch, v, pr, i, 0:128] = x8[row, pr*128+c]; col 128 = 1.0.
    """
    f8np = dt.np(dt.float8e4)
    arr = x8k.reshape(N_CHUNKS, V_PER_CHUNK, 2, 128, N_PAIRS, 128)
    out = np.zeros((128, N_CHUNKS, V_PER_CHUNK, N_PAIRS, 2, CPAD), dtype=f8np)
    out[..., 0:128] = arr.transpose(3, 0, 1, 4, 2, 5)
    out[..., 128] = f8np(1.0)
    return np.ascontiguousarray(out.reshape(128, N_CHUNKS * STAGE_FREE))


def kernel(x, gamma, beta, _trace=False):
    bfnp = dt.np(dt.bfloat16)
    f8np = dt.np(dt.float8e4)
    x = np.asarray(x)
    xf = np.ascontiguousarray(x.reshape(M_TOTAL, C), dtype=np.float32)
    xb = xf.astype(bfnp)
    x8 = xf.astype(f8np)
    gamma_row = np.ascontiguousarray(
        np.asarray(gamma, np.float32).reshape(1, C))
    gamma_t = np.ascontiguousarray(
        np.asarray(gamma, np.float32).reshape(N_PAIRS, 128).T)
    beta_t = np.ascontiguousarray(
        np.asarray(beta, np.float32).reshape(N_PAIRS, 128).T)

    consts = _const_inputs()
    in_maps = []
    for k in range(N_CORES):
        sl = slice(k * M_LOC, (k + 1) * M_LOC)
        m = {"xstat_t": _pack_xstat(x8[sl]),
             "xcol": np.ascontiguousarray(xb[sl].T),
             "gamma_row": gamma_row, "gamma_t": gamma_t, "beta_t": beta_t}
        m.update(consts)
        in_maps.append(m)

    nc = _get_nc()
    kw = {}
    import os as _os
    if _trace and _os.environ.get("DBN_TRACE_ALL"):
        kw["trace_cores"] = list(range(N_CORES))
    res = bass_utils.run_bass_kernel_spmd(
        nc, in_maps, core_ids=list(range(N_CORES)), trace=_trace, **kw)
    out = np.empty((M_TOTAL, C), dtype=np.float32)
    for k in range(N_CORES):
        out[k * M_LOC:(k + 1) * M_LOC] = \
            res.results[k]["yt"].T.astype(np.float32)
    out = out.reshape(N, H, W, C)
    if _trace:
        _CACHED["last_results"] = res
    return out


# revision 41
# speedup vs baseline: 2.0747x; 2.0747x over previous
"""DecorrelatedBN (ZCA whitening) Trainium2 Bass kernel — 8-core data-parallel.

Problem: x [64,32,32,512] f32, NHWC, channel groups of m=64 (G=8 groups).
  out = ((x - mean) @ P) * gamma + beta,  P = (sigma + eps*I)^(-1/2) per group.

v9 design (from v8). Critical-path levers found in the v8 trace:
  - CC-core init barrier (~45us) attaches to the FIRST collective doorbell.
    v8 rang it at ~20.5us (dummy waited on const-DMA chain); v9's dummy
    AllReduce has NO input dependency (uninitialized internal DRAM tile,
    output unused) so the doorbell rings right after the engine preamble
    (~10us) and the barrier fully overlaps phase A.
  - Each sub-256KB AllReduce is latency-bound (~14-15us on the CC stream).
    v8 ran two sigma halves (29us serial); v9 accumulates all 64 row-tiles
    in one PSUM group and runs ONE AllReduce. The partial per-pair mean
    column is also transposed to a row and packed (scaled 1/M) into the
    payload pre-AR, removing one serial post-AR PE round (the mu mu^T
    outer product needs mu as a row).
  - statistics: host pre-tiles an fp8-e4m3 copy of x into the exact SBUF
    stage layout [128p, chunk, super, pair, 2, 144] with a ones column at
    offset 128 (pad to 144 keeps the DoubleRow LDWEIGHTS subtile step%16==0
    ISA rule). Sigma is 128 DoubleRow matmuls and the channel mean
    accumulates in PSUM column 128 for free. Stage chunks post BEFORE the
    xT prefetch so stats never starve (q1 is HBM-bound at ~358GB/s).
  - P = A^(-1/2): deg-4 minimax poly via Paterson-Stockmeyer (2 PE rounds
    vs 4 Horner) + 2 coupled Newton-Schulz (numpy-validated 6.1e-3 vs the
    2e-2 gate; eigenvalues span [0.057, 2.03]).
  - apply: bf16 from host-supplied channel-major xcol, stationary = P' per
    pair, 64 x 512-wide matmuls into [128,2048] PSUM units; each unit's
    eviction is split DVE (cols 0:1024) + ACT (1024:2048) so both engines
    run per unit and PSUM banks free ~1us earlier. Output is transposed
    (yt [512,8192] bf16), host untransposes/upcasts.
"""
import sys

sys.path.insert(0, "/opt/trn_rl_repo")

import numpy as np
import concourse.bass as bass
import concourse.bacc as bacc
import concourse.tile as tile
import concourse.mybir as mybir
from concourse import bass_utils

dt = mybir.dt
Alu = mybir.AluOpType
Act = mybir.ActivationFunctionType
PerfMode = mybir.MatmulPerfMode

VARIANT = "v9"

# Problem constants (hardcoded per harness contract)
N, H, W, C = 64, 32, 32, 512
M_TOTAL = N * H * W          # 65536 rows
N_CORES = 8
M_LOC = M_TOTAL // N_CORES   # 8192 rows per core
GROUP = 64                   # channels per whitening group
N_PAIRS = 4                  # 8 groups packed as 4 pairs of [128,128] blocks
CPAD = 144                   # 128 chans + ones col + pad (step%16==0)

ROWS_PER_TILE = 128
N_TILES = M_LOC // ROWS_PER_TILE      # 64 row-tiles per core
TILES_PER_CHUNK = 16                  # 16 tiles per input DMA
N_CHUNKS = N_TILES // TILES_PER_CHUNK # 4 chunks
V_PER_CHUNK = TILES_PER_CHUNK // 2    # 8 DoubleRow supertiles per chunk
STAGE_FREE = V_PER_CHUNK * N_PAIRS * 2 * CPAD   # 9216 fp8 bytes/partition
N_SUPER = N_CHUNKS * V_PER_CHUNK      # 32 supertiles
NS_ITERS = 2
ROWS_PER_UNIT = 1024                  # apply-phase evict/DMA unit (2 banks)
N_UNITS = M_LOC // ROWS_PER_UNIT      # 8 units per pair

# AR payload layout: [128, AR_COLS] f32
#   cols 0:256    = 8 diag 64x64 sigma blocks (2 per pair, packed)
#   cols 256:260  = per-pair mean columns (raw sums)
AR_COLS = 260

# "all8": 8-rank dummy; "self": per-core groups; "pairs": 2-rank groups
DUMMY_MODE = "self"

# "cc": CC-core AllReduce (init barrier ~45us + ~19us dummy + ~19us AR all
#       serial on the CC stream). "rdma": no collectives at all — each core
#       broadcasts its partial over SDMA to all 7 peers' SBUF (XOR-symmetric
#       slots, one-shot) and sums locally; relies on launch skew << ~20us
#       (observed peer skew at the CC rendezvous is ~5us).
AR_MODE = "cc"

# degree-4 minimax-relative fit of a^-1/2 on [0.03, 2.4] (Lawson); with
# 2 coupled NS iterations: rel err ~6e-4 on the data eigenvalue range
# [0.057, 2.03] (deterministic seed).
POLY_COEF = [4.858203701346275, -13.706787063800203, 16.713432649944906,
             -8.387599448841533, 1.462158293274531]

_CACHED = {}


def _build_bass():
    nc = bacc.Bacc("TRN2", target_bir_lowering=False, debug=False,
                   num_devices=N_CORES,
                   num_swdge_queues=4 if AR_MODE == "rdma" else 1)
    f32 = dt.float32
    bf16 = dt.bfloat16
    f8 = dt.float8e4

    xstat_t = nc.dram_tensor("xstat_t", [128, N_CHUNKS * STAGE_FREE], f8,
                             kind="ExternalInput").ap()
    xcol = nc.dram_tensor("xcol", [C, M_LOC], bf16, kind="ExternalInput").ap()
    gamma_row = nc.dram_tensor("gamma_row", [1, C], f32, kind="ExternalInput").ap()
    gamma_t = nc.dram_tensor("gamma_t", [128, N_PAIRS], f32, kind="ExternalInput").ap()
    beta_t = nc.dram_tensor("beta_t", [128, N_PAIRS], f32, kind="ExternalInput").ap()
    ident = nc.dram_tensor("ident", [128, 128], f32, kind="ExternalInput").ap()
    eye15 = nc.dram_tensor("eye15", [128, 128], f32, kind="ExternalInput").ap()
    ones_row = nc.dram_tensor("ones_row", [1, 128], f32, kind="ExternalInput").ap()
    yt = nc.dram_tensor("yt", [C, M_LOC], bf16, kind="ExternalOutput").ap()

    if DUMMY_MODE == "self":
        dummy_groups = [[k] for k in range(N_CORES)]
    elif DUMMY_MODE == "pairs":
        dummy_groups = [[2 * k, 2 * k + 1] for k in range(N_CORES // 2)]
    else:
        dummy_groups = [list(range(N_CORES))]

    with tile.TileContext(nc) as tc:
        with (
            tc.tile_pool(name="const", bufs=1) as constp,
            tc.tile_pool(name="resid", bufs=1) as residp,
            tc.tile_pool(name="small", bufs=1) as smallp,
            tc.tile_pool(name="dram", bufs=1, space="DRAM") as dramp,
        ):
            # dummy collective, zero dependencies; output never read so no
            # engine queue blocks on its completion. In cc mode it rings
            # the CC doorbell right after the preamble so the ~45us CC-core
            # init barrier overlaps phase A. In rdma mode it is emitted
            # AFTER the exchange's critical section instead (the section's
            # global-clock entry snapshot would otherwise gate the exchange
            # trigger on the dummy's ~84us CC completion) — still load-
            # bearing there: has_collectives=True makes PJRT gang-launch
            # the 8 cores (otherwise launches stagger ~1ms/core through
            # the axon tunnel and the peer-exchange waits eat the skew).
            dummy_in = dramp.tile([1, N_PAIRS], f32, name="dummy_in")
            dummy_out = dramp.tile([1, N_PAIRS], f32, name="dummy_out")

            def emit_dummy():
                nc.gpsimd.collective_compute(
                    "AllReduce", Alu.add,
                    replica_groups=dummy_groups if AR_MODE == "cc"
                    else [list(range(N_CORES))],
                    ins=[dummy_in.opt()], outs=[dummy_out.opt()],
                )

            if AR_MODE == "cc":
                emit_dummy()

            # ---- constants to SBUF ----
            id_sb = constp.tile([128, 128], f32, name="id_sb")
            eye15_sb = constp.tile([128, 128], f32, name="eye15_sb")
            onesr_sb = constp.tile([1, 128], f32, name="onesr_sb")
            grow_sb = constp.tile([1, C], f32, name="grow_sb")
            gt_sb = constp.tile([128, N_PAIRS], f32, name="gt_sb")
            bt_sb = constp.tile([128, N_PAIRS], f32, name="bt_sb")

            # resident channel-major x, one tile per 128-channel block so
            # phase B's per-pair deps attach to exactly one prefetch DMA
            xT = [residp.tile([128, M_LOC], bf16, name=f"xT{b}")
                  for b in range(N_PAIRS)]

            # AR staging
            sig_full = smallp.tile([128, AR_COLS], f32, name="sig_full")
            ar_in = dramp.tile([128, AR_COLS], f32, name="ar_in")
            ar_out = dramp.tile([128, AR_COLS], f32, name="ar_out")
            sigsum = smallp.tile([128, AR_COLS], f32, name="sigsum")
            c0I_sb = smallp.tile([128, 128], f32, name="c0I_sb")

            # PE warmup: HAM clock-gate needs sustained matmul activity
            warm_sb = constp.tile([128, 512], bf16, name="warm_sb")

            with (
                tc.tile_pool(name="instage", bufs=1) as inp,
                tc.tile_pool(name="sigps", bufs=1, space="PSUM") as sigpp,
            ):
                sig_ps = [sigpp.tile([128, CPAD - 15], f32, name=f"sig{p}",
                                     tag=f"sig{p}") for p in range(N_PAIRS)]
                stages = [inp.tile([128, V_PER_CHUNK, N_PAIRS, 2, CPAD], f8,
                                   name=f"stage{ch}", tag=f"instage{ch}")
                          for ch in range(N_CHUNKS)]

                # post input DMAs up-front: consts go on the SCALAR queue
                # (their many small per-partition descriptors would clog q1
                # ahead of the stats chunks); stats chunks first on q1
                # (phase A is PE-bound and must never starve). The xT posts
                # are emitted AFTER the rdma critical section (in rdma mode)
                # so its entry snapshot does not gate the exchange trigger
                # on the xT loads; they still stream right behind the
                # chunks on q1.
                nc.scalar.dma_start(id_sb[:], ident[:])
                nc.scalar.dma_start(eye15_sb[:], eye15[:])
                nc.scalar.dma_start(onesr_sb[:], ones_row[:])
                nc.scalar.dma_start(grow_sb[:], gamma_row[:])
                nc.scalar.dma_start(gt_sb[:], gamma_t[:])
                nc.scalar.dma_start(bt_sb[:], beta_t[:])
                for ch in range(N_CHUNKS):
                    nc.sync.dma_start(
                        stages[ch][:],
                        xstat_t[:, ch * STAGE_FREE:(ch + 1) * STAGE_FREE])
                if AR_MODE == "cc":
                    for b in range(N_PAIRS):
                        nc.sync.dma_start(xT[b][:],
                                          xcol[b * 128:(b + 1) * 128, :])

                nc.vector.memset(warm_sb[:], 0.5)

                # PE warmup while chunk 0 streams in
                with tc.tile_pool(name="warmps", bufs=1, space="PSUM") as warmpp:
                    warm_ps = warmpp.tile([128, 512], f32, name="warm_ps")
                    for _ in range(12):
                        nc.tensor.matmul(warm_ps[:], warm_sb[:, 0:128],
                                         warm_sb[:], start=True, stop=True)

                # ============ Phase A: stats (fp8 DoubleRow) ============
                # stage layout [128, super, pair, 2, 144]: data cols 0:128,
                # ones col 128 (mean accumulates in sigma PSUM col 128), pad
                # to 144 for the dual-fp8 LDWEIGHTS step%16 rule. ONE PSUM
                # accumulation group over all 32 supertiles.
                for ch in range(N_CHUNKS):
                    for v in range(V_PER_CHUNK):
                        s = ch * V_PER_CHUNK + v
                        for p in range(N_PAIRS):
                            nc.tensor.matmul(
                                sig_ps[p][:],
                                stages[ch][:, v, p, :, 0:128],
                                stages[ch][:, v, p, :, 0:129],
                                start=(s == 0), stop=(s == N_SUPER - 1),
                                perf_mode=PerfMode.DoubleRow)

                # evac: pack diag blocks + raw mean cols into sig_full
                for p in range(N_PAIRS):
                    nc.scalar.copy(sig_full[0:64, p * 64:(p + 1) * 64],
                                   sig_ps[p][0:64, 0:64])
                    nc.scalar.copy(sig_full[64:128, p * 64:(p + 1) * 64],
                                   sig_ps[p][64:128, 64:128])
                    nc.scalar.copy(sig_full[:, 256 + p:257 + p],
                                   sig_ps[p][:, 128:129])

            if AR_MODE == "cc":
                # ONE AllReduce for everything
                nc.scalar.dma_start(ar_in[:], sig_full[:])
                nc.gpsimd.collective_compute(
                    "AllReduce", Alu.add,
                    replica_groups=[list(range(N_CORES))],
                    ins=[ar_in.opt()], outs=[ar_out.opt()],
                )
                nc.scalar.dma_start(sigsum[:], ar_out[:])
            else:
                # one-shot SDMA all-exchange: 7 single-slot relative
                # broadcasts, slot k -> peer (tpb ^ k), landing in
                # recv[k-1] (XOR-symmetric: at receiver q, slot k's writer
                # is q^k, so every peer lands in a distinct slot; slot k
                # naturally satisfies the D2D-lane rule). Direct one-hop
                # paths are optimal under launch skew (~50us cross-die):
                # core 0's wait = max over peers of skew + peer-stats +
                # transfer; a multi-round ring relays the slowest core's
                # data through extra hops. All control on gpsimd inside a
                # critical section (Tile would otherwise gate the adds on
                # DESC-GEN completion, which merely looks like the writer).
                rd_mono = nc.monotonic_semaphore(0)
                rd_local = nc.alloc_semaphore("rd_local_sem")
                rd_prep = nc.alloc_semaphore("rd_prep_sem")
                recv = [smallp.tile([128, AR_COLS], f32, name=f"rdrecv{k}")
                        for k in range(1, N_CORES)]
                with tc.tile_critical(name="rdma_ar"):
                    for k in range(1, N_CORES):
                        rdests = [None] * N_CORES
                        rdests[k] = (0, k)
                        q = (k - 1) % 4
                        nc.gpsimd.remote_dma_broadcast(
                            recv[k - 1][:], sig_full[:], rd_mono.sem(),
                            rd_local, rdests=rdests,
                            queue_num=q).then_inc(rd_prep, 1)
                    tc.wait_critical_data_deps()
                    nc.gpsimd.wait_ge(rd_prep, N_CORES - 1)
                    for q in range(4):
                        nc.gpsimd.trigger_dma(count=2 if q < 3 else 1,
                                              queue_num=q)
                    # wait for all 7 peers' payloads (2 lane-incs each) on
                    # gpsimd (owns the monotonic counter register), then
                    # gate the DVE reduction (raw order holds in-section)
                    rd_gate = nc.alloc_semaphore("rd_gate_sem")
                    rd_mono.wait_inc(2 * (N_CORES - 1)).then_inc(rd_gate, 1)
                    nc.vector.wait_ge(rd_gate, 1)
                    nc.vector.tensor_tensor(sigsum[:], sig_full[:],
                                            recv[0][:], op=Alu.add)
                    for j in range(1, N_CORES - 1):
                        nc.vector.tensor_tensor(sigsum[:], sigsum[:],
                                                recv[j][:], op=Alu.add)
                # fake dep: write dummy_in from sigsum so Tile schedules
                # the dummy AFTER the exchange (a dep-free collective gets
                # scheduled early and its completion-increment lands in the
                # critical section's entry snapshot, gating the trigger on
                # the ~80us CC dummy completion)
                nc.scalar.dma_start(dummy_in[:], sigsum[0:1, 0:N_PAIRS])
                emit_dummy()
                for b in range(N_PAIRS):
                    nc.sync.dma_start(xT[b][:],
                                      xcol[b * 128:(b + 1) * 128, :])

            # gamma replicate (depends only on gamma): runs during the AR
            grep_sb = smallp.tile([128, C], f32, name="grep_sb")
            with tc.tile_pool(name="grepps", bufs=1, space="PSUM") as greppp:
                grep_ps = greppp.tile([128, C], f32, name="grep_ps")
                nc.tensor.matmul(grep_ps[:], onesr_sb[:], grow_sb[:],
                                 start=True, stop=True)
                nc.scalar.copy(grep_sb[:], grep_ps[:])
            nc.vector.tensor_scalar_mul(c0I_sb[:], id_sb[:],
                                        float(POLY_COEF[0]))

            # keep the PE/HAM clock warm through the AllReduce wait
            with tc.tile_pool(name="warmps2", bufs=1, space="PSUM") as warmpp2:
                warm2_ps = warmpp2.tile([128, 512], f32, name="warm2_ps")
                for _ in range(24):
                    nc.tensor.matmul(warm2_ps[:, 0:256], warm_sb[:, 0:128],
                                     warm_sb[:, 0:256], start=True, stop=True)

            # ================= small-matrix phase (all fp32) ==============
            with tc.tile_pool(name="nsps", bufs=2, space="PSUM") as nspp:
                mu_row = smallp.tile([1, C], f32, name="mu_row")
                mu_col = smallp.tile([128, N_PAIRS], f32, name="mu_col")
                for p in range(N_PAIRS):
                    nc.vector.tensor_scalar_mul(
                        mu_col[:, p:p + 1], sigsum[:, 256 + p:257 + p],
                        1.0 / M_TOTAL)
                    mr_ps = nspp.tile([1, 128], f32, tag="ns2")
                    nc.tensor.matmul(mr_ps[:], mu_col[:, p:p + 1], id_sb[:],
                                     start=True, stop=True)
                    nc.vector.tensor_copy(mu_row[0:1, p * 128:(p + 1) * 128],
                                          mr_ps[:])

                Y_sb = [smallp.tile([128, 128], f32, name=f"Y{p}")
                        for p in range(N_PAIRS)]
                Z_sb = [smallp.tile([128, 128], f32, name=f"Z{p}")
                        for p in range(N_PAIRS)]
                B_sb = [smallp.tile([128, 128], f32, name=f"B{p}")
                        for p in range(N_PAIRS)]
                A_sb = [smallp.tile([128, 128], f32, name=f"A{p}")
                        for p in range(N_PAIRS)]
                A2_sb = [smallp.tile([128, 128], f32, name=f"A2_{p}")
                         for p in range(N_PAIRS)]
                T1_sb = [smallp.tile([128, 128], f32, name=f"T1_{p}")
                         for p in range(N_PAIRS)]
                T0_sb = [smallp.tile([128, 128], f32, name=f"T0_{p}")
                        for p in range(N_PAIRS)]
                Pb_sb = [smallp.tile([128, 128], bf16, name=f"Pb{p}")
                         for p in range(N_PAIRS)]

                # A_p = blockdiag(sigsum/M - mu mu^T)   (eps dropped: <1e-4)
                for p in range(N_PAIRS):
                    murow = mu_row[0:1, p * 128:(p + 1) * 128]
                    outer_ps = nspp.tile([128, 128], f32, tag="ns0")
                    nc.tensor.matmul(outer_ps[:], murow, murow,
                                     start=True, stop=True)
                    nc.vector.memset(A_sb[p][:], 0.0)
                    nc.vector.scalar_tensor_tensor(
                        A_sb[p][0:64, 0:64], sigsum[0:64, p * 64:(p + 1) * 64],
                        1.0 / M_TOTAL, outer_ps[0:64, 0:64],
                        op0=Alu.mult, op1=Alu.subtract)
                    nc.vector.scalar_tensor_tensor(
                        A_sb[p][64:128, 64:128],
                        sigsum[64:128, p * 64:(p + 1) * 64],
                        1.0 / M_TOTAL, outer_ps[64:128, 64:128],
                        op0=Alu.mult, op1=Alu.subtract)

                # polynomial init via Paterson-Stockmeyer (2 PE rounds):
                #   A2 = A@A
                #   T1 = c3*A + c4*A2 ; T0 = c1*A + (c2*A2 + c0*I)
                #   Z0 = T1@A2 + T0
                c1, c2, c3, c4 = (float(POLY_COEF[1]), float(POLY_COEF[2]),
                                  float(POLY_COEF[3]), float(POLY_COEF[4]))
                for p in range(N_PAIRS):
                    a2_ps = nspp.tile([128, 128], f32, tag="ns0")
                    nc.tensor.matmul(a2_ps[:], A_sb[p][:], A_sb[p][:],
                                     start=True, stop=True)
                    nc.vector.tensor_copy(A2_sb[p][:], a2_ps[:])
                for p in range(N_PAIRS):
                    nc.vector.tensor_scalar_mul(T1_sb[p][:], A2_sb[p][:], c4)
                    nc.vector.scalar_tensor_tensor(
                        T1_sb[p][:], A_sb[p][:], c3, T1_sb[p][:],
                        op0=Alu.mult, op1=Alu.add)
                    nc.vector.scalar_tensor_tensor(
                        T0_sb[p][:], A2_sb[p][:], c2, c0I_sb[:],
                        op0=Alu.mult, op1=Alu.add)
                    nc.vector.scalar_tensor_tensor(
                        T0_sb[p][:], A_sb[p][:], c1, T0_sb[p][:],
                        op0=Alu.mult, op1=Alu.add)
                for p in range(N_PAIRS):
                    z_ps = nspp.tile([128, 128], f32, tag="ns1")
                    nc.tensor.matmul(z_ps[:], T1_sb[p][:], A2_sb[p][:],
                                     start=True, stop=True)
                    nc.vector.tensor_tensor(Z_sb[p][:], z_ps[:], T0_sb[p][:],
                                            op=Alu.add)
                # Y0 = A @ Z0
                for p in range(N_PAIRS):
                    y_ps = nspp.tile([128, 128], f32, tag="ns1")
                    nc.tensor.matmul(y_ps[:], A_sb[p][:], Z_sb[p][:],
                                     start=True, stop=True)
                    nc.scalar.copy(Y_sb[p][:], y_ps[:])

                # coupled Newton-Schulz (2 iters, unrolled): W=Z@Y;
                # B=1.5I-0.5W; Y=Y@B; Z=B@Z. The LAST iteration runs its
                # matmuls in bf16 (single-pass PE vs fp32's LOW/HIGH double
                # pass; numpy-validated 7.4e-3 vs the 2e-2 gate — fp32 is
                # only load-bearing for A/poly where 1/lambda_min amplifies
                # rounding). bias needs mp = Z_final @ mu = B2 @ (Z1 @ mu):
                # v rides the last W round, mp rides the last Z round.
                bias_col = smallp.tile([128, N_PAIRS], f32, name="bias_col")
                tmp_col = smallp.tile([128, N_PAIRS], f32, name="tmp_col")
                vb_col = smallp.tile([128, N_PAIRS], bf16, name="vb_col")
                mub_col = smallp.tile([128, N_PAIRS], bf16, name="mub_col")
                Yb_sb = [smallp.tile([128, 128], bf16, name=f"Yb{p}")
                         for p in range(N_PAIRS)]
                Zb_sb = [smallp.tile([128, 128], bf16, name=f"Zb{p}")
                         for p in range(N_PAIRS)]
                Bb_sb = [smallp.tile([128, 128], bf16, name=f"Bb{p}")
                         for p in range(N_PAIRS)]
                nc.vector.tensor_copy(mub_col[:], mu_col[:])
                # ---- iteration 1 (fp32, evict to bf16) ----
                for p in range(N_PAIRS):
                    w_ps = nspp.tile([128, 128], f32, tag="ns0")
                    nc.tensor.matmul(w_ps[:], Z_sb[p][:], Y_sb[p][:],
                                     start=True, stop=True)
                    nc.vector.scalar_tensor_tensor(
                        B_sb[p][:], w_ps[:], -0.5, eye15_sb[:],
                        op0=Alu.mult, op1=Alu.add)
                for p in range(N_PAIRS):
                    y_ps = nspp.tile([128, 128], f32, tag="ns1")
                    z_ps = nspp.tile([128, 128], f32, tag="ns2")
                    nc.tensor.matmul(y_ps[:], Y_sb[p][:], B_sb[p][:],
                                     start=True, stop=True)
                    nc.scalar.copy(Yb_sb[p][:], y_ps[:])
                    nc.tensor.matmul(z_ps[:], B_sb[p][:], Z_sb[p][:],
                                     start=True, stop=True)
                    nc.vector.tensor_copy(Zb_sb[p][:], z_ps[:])
                # ---- iteration 2 (bf16 matmuls) ----
                for p in range(N_PAIRS):
                    w_ps = nspp.tile([128, 128], f32, tag="ns0")
                    nc.tensor.matmul(w_ps[:], Zb_sb[p][:], Yb_sb[p][:],
                                     start=True, stop=True)
                    v_ps = nspp.tile([128, 1], f32, tag="ns2")
                    nc.tensor.matmul(v_ps[:], Zb_sb[p][:],
                                     mub_col[:, p:p + 1],
                                     start=True, stop=True)
                    nc.vector.tensor_copy(vb_col[:, p:p + 1], v_ps[:])
                    nc.vector.scalar_tensor_tensor(
                        Bb_sb[p][:], w_ps[:], -0.5, eye15_sb[:],
                        op0=Alu.mult, op1=Alu.add)
                for p in range(N_PAIRS):
                    z_ps = nspp.tile([128, 128], f32, tag="ns1")
                    nc.tensor.matmul(z_ps[:], Bb_sb[p][:], Zb_sb[p][:],
                                     start=True, stop=True)
                    # gamma fold straight from PSUM: P'_bf = Z2 .* gamma_rep
                    nc.vector.tensor_tensor(
                        Pb_sb[p][:], z_ps[:],
                        grep_sb[:, p * 128:(p + 1) * 128], op=Alu.mult)
                    mp_ps = nspp.tile([128, 1], f32, tag="ns0")
                    nc.tensor.matmul(mp_ps[:], Bb_sb[p][:],
                                     vb_col[:, p:p + 1],
                                     start=True, stop=True)
                    nc.vector.tensor_scalar(
                        tmp_col[:, p:p + 1], mp_ps[:],
                        gt_sb[:, p:p + 1], None, op0=Alu.mult)
                    nc.vector.scalar_tensor_tensor(
                        bias_col[:, p:p + 1], tmp_col[:, p:p + 1],
                        -1.0, bt_sb[:, p:p + 1],
                        op0=Alu.mult, op1=Alu.add)

            # ================= Phase B: apply =================
            # [128,1024] PSUM units (2 banks) x 4 bufs -> 4 units in flight;
            # whole-unit evictions alternate DVE/ACT (PSUM reads run at
            # ~0.5 elem/cyc/engine, so two engines on one unit just split
            # the same bandwidth — alternating engines across units gives
            # the same throughput with fewer dependency edges).
            with (
                tc.tile_pool(name="outstage", bufs=4) as outp,
                tc.tile_pool(name="whps", bufs=4, space="PSUM") as whpp,
            ):
                ucount = 0
                for p in range(N_PAIRS):
                    for un in range(N_UNITS):
                        r0 = un * ROWS_PER_UNIT
                        wh = whpp.tile([128, ROWS_PER_UNIT], f32, tag="whps")
                        for h in range(ROWS_PER_UNIT // 512):
                            nc.tensor.matmul(
                                wh[:, h * 512:(h + 1) * 512],
                                Pb_sb[p][:],
                                xT[p][:, r0 + h * 512: r0 + (h + 1) * 512],
                                start=True, stop=True)
                        ostage = outp.tile([128, ROWS_PER_UNIT], bf16,
                                           tag="outstage")
                        if ucount % 2 == 0:
                            nc.vector.tensor_scalar(
                                ostage[:], wh[:], bias_col[:, p:p + 1], None,
                                op0=Alu.add)
                        else:
                            nc.scalar.activation(
                                ostage[:], wh[:], Act.Identity,
                                bias=bias_col[:, p:p + 1], scale=1.0)
                        nc.sync.dma_start(
                            yt[p * 128:(p + 1) * 128, r0:r0 + ROWS_PER_UNIT],
                            ostage[:])
                        ucount += 1

    nc.compile()
    return nc


def _get_nc():
    if "nc" not in _CACHED:
        _CACHED["nc"] = _build_bass()
    return _CACHED["nc"]


def _const_inputs():
    if "consts" not in _CACHED:
        ident = np.eye(128, dtype=np.float32)
        _CACHED["consts"] = {
            "ident": ident,
            "eye15": (1.5 * ident).astype(np.float32),
            "ones_row": np.ones((1, 128), dtype=np.float32),
        }
    return _CACHED["consts"]


def _pack_xstat(x8k):
    """[8192, 512] fp8 -> pre-tiled stage mirror [128, N_CHUNKS*STAGE_FREE].

    row = ch*2048 + v*256 + i*128 + p; channel = pr*128 + c;
    dest[p, ch, v, pr, i, 0:128] = x8[row, pr*128+c]; col 128 = 1.0.
    """
    f8np = dt.np(dt.float8e4)
    arr = x8k.reshape(N_CHUNKS, V_PER_CHUNK, 2, 128, N_PAIRS, 128)
    out = np.zeros((128, N_CHUNKS, V_PER_CHUNK, N_PAIRS, 2, CPAD), dtype=f8np)
    out[..., 0:128] = arr.transpose(3, 0, 1, 4, 2, 5)
    out[..., 128] = f8np(1.0)
    return np.ascontiguousarray(out.reshape(128, N_CHUNKS * STAGE_FREE))


def kernel(x, gamma, beta, _trace=False):
    bfnp = dt.np(dt.bfloat16)
    f8np = dt.np(dt.float8e4)
    x = np.asarray(x)
    xf = np.ascontiguousarray(x.reshape(M_TOTAL, C), dtype=np.float32)
    xb = xf.astype(bfnp)
    x8 = xf.astype(f8np)
    gamma_row = np.ascontiguousarray(
        np.asarray(gamma, np.float32).reshape(1, C))
    gamma_t = np.ascontiguousarray(
        np.asarray(gamma, np.float32).reshape(N_PAIRS, 128).T)
    beta_t = np.ascontiguousarray(
        np.asarray(beta, np.float32).reshape(N_PAIRS, 128).T)

    consts = _const_inputs()
    in_maps = []
    for k in range(N_CORES):
        sl = slice(k * M_LOC, (k + 1) * M_LOC)
        m = {"xstat_t": _pack_xstat(x8[sl]),
             "xcol": np.ascontiguousarray(xb[sl].T),
             "gamma_row": gamma_row, "gamma_t": gamma_t, "beta_t": beta_t}
        m.update(consts)
        in_maps.append(m)

    nc = _get_nc()
    kw = {}
    import os as _os
    if _trace and _os.environ.get("DBN_TRACE_ALL"):
        kw["trace_cores"] = list(range(N_CORES))
    res = bass_utils.run_bass_kernel_spmd(
        nc, in_maps, core_ids=list(range(N_CORES)), trace=_trace, **kw)
    out = np.empty((M_TOTAL, C), dtype=np.float32)
    for k in range(N_CORES):
        out[k * M_LOC:(k + 1) * M_LOC] = \
            res.results[k]["yt"].T.astype(np.float32)
    out = out.reshape(N, H, W, C)
    if _trace:
        _CACHED["last_results"] = res
    return out
